# revision 15
# baseline (speedup 1.0000x reference)
"""BrainAgeGATv2 on 8 Trainium2 NeuronCores (Bass/Tile).

Strategy: shard dst-nodes across 8 cores (5120 padded nodes each, 40
windows of 128). Host sorts edges (+self loops) by dst window, splits each
window's edges by src table-half (int16 gather index limit), pads each part
to 128-edge tiles. Per layer: local transforms produce xl/xr row tables
(biases fold into the edge-score matmul / cancel in BatchNorm); AllGather
distributes the xl row table; the edge phase gathers xl (transposed bf16 +
row bf16) and xr (SBUF-source transposed), assembles GATv2 scores on
PE/ACT, softmax via exp without max-subtraction (validated safe range) with
normalization deferred to per-window den division; aggregation via onehot
selection matmuls accumulated in PSUM. BN stats AllReduce, pooling via
host-built membership matmul AllReduce, MLP head replicated per core.
"""
import numpy as np
import ml_dtypes

import concourse.bass as bass
import concourse.bacc as bacc
import concourse.mybir as mybir
import concourse.tile as tile
from concourse import bass_utils

F32 = mybir.dt.float32
BF16 = mybir.dt.bfloat16
I16 = mybir.dt.int16

N = 40000
E = 400000
B = 100
H, C = 8, 16
HC = 128
NEG = 0.2
EPS = 1e-5
NCORES = 8
NPC = 5120            # padded nodes per core
NPAD = NCORES * NPC   # 40960
WIN = 128
WINS = NPC // WIN     # 40
HALF = 32768          # src table split (int16 index limit)
DEN_EPS = 1e-30

bf = ml_dtypes.bfloat16


def _wrap_idx(idx_i16):
    """dma_gather index layout: i = idxs[i%16, i//16], replicated to 128 parts."""
    blk = idx_i16.reshape(-1, 16).T.copy()          # [16, n/16]
    return np.tile(blk, (8, 1))                     # [128, n/16]


def preprocess(x, edge_index, edge_attr, batch, global_features):
    src = np.concatenate([edge_index[0], np.arange(N, dtype=np.int64)])
    dst = np.concatenate([edge_index[1], np.arange(N, dtype=np.int64)])
    ea = np.concatenate([edge_attr[:, 0].astype(np.float64),
                         np.full(N, edge_attr.astype(np.float64).mean())]
                        ).astype(np.float32)

    w = dst // WIN                       # global window 0..319
    hi = (src >= HALF).astype(np.int64)
    order = np.lexsort((hi, w))
    s_src, s_dst, s_ea, s_w, s_hi = (a[order] for a in (src, dst, ea, w, hi))

    nwin = NPAD // WIN                   # 320
    key = s_w * 2 + s_hi
    bounds = np.searchsorted(key, np.arange(2 * nwin + 1))
    lo_counts = bounds[1::2] - bounds[0:-1:2]
    hi_counts = bounds[2::2] - bounds[1::2]
    TL = int(np.ceil(lo_counts.max() / 128))
    TH = int(max(1, np.ceil(hi_counts.max() / 128)))
    TW = TL + TH
    SLOTS_W = TW * 128
    EPC = WINS * SLOTS_W                 # padded slots per core

    # per-slot arrays, all cores: slot s of core c
    g_src = np.zeros((NCORES, EPC), np.int64)       # table-relative src idx
    g_xr = np.zeros((NCORES, EPC), np.int64)
    g_ea = np.zeros((NCORES, EPC), np.float32)
    g_dr = np.full((NCORES, EPC), -1.0, np.float32)  # dst_rel or -1

    for gw in range(nwin):
        c, lw = divmod(gw, WINS)
        base = lw * SLOTS_W
        lo0, lo1 = bounds[2 * gw], bounds[2 * gw + 1]
        hi1 = bounds[2 * gw + 2]
        nlo, nhi = lo1 - lo0, hi1 - lo1
        sl = slice(base, base + nlo)
        g_src[c, sl] = s_src[lo0:lo1]
        g_xr[c, sl] = s_dst[lo0:lo1] - c * NPC
        g_ea[c, sl] = s_ea[lo0:lo1]
        g_dr[c, sl] = s_dst[lo0:lo1] - gw * WIN
        base_h = base + TL * 128
        sh = slice(base_h, base_h + nhi)
        g_src[c, sh] = s_src[lo1:hi1] - HALF
        g_xr[c, sh] = s_dst[lo1:hi1] - c * NPC
        g_ea[c, sh] = s_ea[lo1:hi1]
        g_dr[c, sh] = s_dst[lo1:hi1] - gw * WIN

    # chunk structure (uniform across cores/windows): list of (tile0, ntiles, is_hi)
    chunks = []
    t = 0
    while t < TL:
        n = min(4, TL - t)
        chunks.append((t, n))
        t += n
    while t < TW:
        n = min(4, TW - t)
        chunks.append((t, n))
        t += n

    # pooling membership (value 1/cnt), general sorted batch
    cnt = np.bincount(batch, minlength=B).astype(np.float64)
    inv = np.where(cnt > 0, 1.0 / np.maximum(cnt, 1.0), 0.0)
    S = np.zeros((NCORES, NPC, B), np.float32)
    node_ids = np.arange(N)
    for c in range(NCORES):
        lo, hiN = c * NPC, min((c + 1) * NPC, N)
        loc = node_ids[lo:hiN] - lo
        S[c, loc, batch[lo:hiN]] = inv[batch[lo:hiN]]

    per_core = []
    for c in range(NCORES):
        xs = np.zeros((NPC, 4), np.float32)
        lo, hiN = c * NPC, min((c + 1) * NPC, N)
        xs[: hiN - lo] = x[lo:hiN]
        drc = g_dr[c].reshape(-1, 128)                  # [ntiles, 128]
        jj = np.arange(128, dtype=np.float32)
        per_core.append(dict(
            x_T=np.ascontiguousarray(xs.T).astype(bf),
            idx_g=_wrap_idx(g_src[c].astype(np.int16)),
            idx_xr=_wrap_idx(g_xr[c].astype(np.int16)),
            ea2=np.stack([g_ea[c], np.ones(EPC, np.float32)]).astype(bf),
            dstrel=np.ascontiguousarray(
                g_dr[c].reshape(-1, 128).T).astype(np.float32),  # [128, ntiles]
            oh_en=(drc[:, :, None] == jj).astype(bf),           # [T, e, n]
            oh_ne=(drc[:, None, :] == jj[:, None]).astype(bf),  # [T, n, e]
            S=S[c].astype(bf),
        ))
    struct = dict(TL=TL, TH=TH, TW=TW, EPC=EPC, chunks=chunks,
                  NTILES=WINS * TW)
    return per_core, struct


def pack_weights(ws):
    att_blk = np.zeros((4, HC, H), np.float32)
    bones = np.zeros((H, HC), np.float32)
    for h in range(H):
        bones[h, 16 * h:16 * h + 16] = 1.0
    for l in range(4):
        for h in range(H):
            att_blk[l, 16 * h:16 * h + 16, h] = ws['att'][l, h]
    eW2 = np.zeros((4, 2, HC), np.float32)
    for l in range(4):
        eW2[l, 0] = ws['We'][l, 0]
        eW2[l, 1] = ws['bl'][l] + ws['br'][l]
    wlr0 = np.concatenate([ws['Wl1'], ws['Wr1']], axis=1)         # [64, 256]
    wlrR = np.concatenate([ws['Wl234'], ws['Wr234']], axis=2)     # [3,128,256]
    return dict(
        Wemb=ws['W_emb'].astype(bf), bemb=ws['b_emb'].reshape(64, 1).astype(np.float32),
        Wlr0=wlr0.astype(bf), WlrR=wlrR.astype(bf),
        eW2=eW2.astype(bf), attblk=att_blk.astype(bf), bones=bones.astype(bf),
        gammaW=ws['gamma'].reshape(4, HC, 1).astype(np.float32),
        betaW=ws['beta'].reshape(4, HC, 1).astype(np.float32),
        mW1=ws['meta_W1'].astype(np.float32), mb1=ws['meta_b1'].reshape(16, 1).astype(np.float32),
        mW2=ws['meta_W2'].astype(np.float32), mb2=ws['meta_b2'].reshape(16, 1).astype(np.float32),
        gW1=ws['graph_W1'].astype(np.float32), gb1=ws['graph_b1'].reshape(16, 1).astype(np.float32),
        gW2=ws['graph_W2'].astype(np.float32), gb2=ws['graph_b2'].reshape(16, 1).astype(np.float32),
        fW1a=ws['fc_W1'][:128].astype(np.float32),
        fW1bm=ws['fc_W1'][128:144].astype(np.float32),
        fW1bg=ws['fc_W1'][144:160].astype(np.float32),
        fb1=ws['fc_b1'].reshape(128, 1).astype(np.float32),
        fW2=ws['fc_W2'].astype(np.float32), fb2=ws['fc_b2'].reshape(64, 1).astype(np.float32),
        fW3=ws['fc_W3'].astype(np.float32),
        fb3=float(np.asarray(ws['fc_b3']).reshape(-1)[0]),
        iota_t=np.tile(np.arange(128, dtype=np.float32), (128, 1)),
        I128=np.eye(128, dtype=np.float32).astype(bf),
    )


def build(st, wp_shapes_only):
    TL, TH, TW, EPC, chunks, NTILES = (st[k] for k in
                                       ('TL', 'TH', 'TW', 'EPC', 'chunks', 'NTILES'))
    RG = [list(range(NCORES))]
    nc = bacc.Bacc("TRN2", target_bir_lowering=False, debug=False,
                   num_devices=NCORES, num_swdge_queues=3)

    def din(name, shape, dt):
        return nc.dram_tensor(name, list(shape), dt, kind="ExternalInput")

    # per-core inputs
    x_T = din("x_T", (4, NPC), BF16)
    idx_g = din("idx_g", (128, EPC // 16), I16)
    idx_xr = din("idx_xr", (128, EPC // 16), I16)
    ea2_d = din("ea2", (2, EPC), BF16)
    dstrel_d = din("dstrel", (128, NTILES), F32)
    S_d = din("S", (NPC, B), BF16)
    oh_en_d = din("oh_en", (NTILES, 128, 128), BF16)
    oh_ne_d = din("oh_ne", (NTILES, 128, 128), BF16)
    gf_T = din("gf_T", (6, B), F32)
    # weights
    Wemb = din("Wemb", (4, 64), BF16)
    bemb = din("bemb", (64, 1), F32)
    Wlr0 = din("Wlr0", (64, 256), BF16)
    WlrR = din("WlrR", (3, 128, 256), BF16)
    eW2_d = din("eW2", (4, 2, HC), BF16)
    attblk_d = din("attblk", (4, HC, H), BF16)
    bones_d = din("bones", (H, HC), BF16)
    gamma_d = din("gammaW", (4, HC, 1), F32)
    beta_d = din("betaW", (4, HC, 1), F32)
    iota_d = din("iota_t", (128, 128), F32)
    I128_d = din("I128", (128, 128), BF16)
    mW1 = din("mW1", (4, 16), F32); mb1 = din("mb1", (16, 1), F32)
    mW2 = din("mW2", (16, 16), F32); mb2 = din("mb2", (16, 1), F32)
    gW1 = din("gW1", (2, 16), F32); gb1 = din("gb1", (16, 1), F32)
    gW2 = din("gW2", (16, 16), F32); gb2 = din("gb2", (16, 1), F32)
    fW1a = din("fW1a", (128, 128), F32)
    fW1bm = din("fW1bm", (16, 128), F32); fW1bg = din("fW1bg", (16, 128), F32)
    fb1 = din("fb1", (128, 1), F32)
    fW2 = din("fW2", (128, 64), F32); fb2 = din("fb2", (64, 1), F32)
    fW3 = din("fW3", (64, 1), F32)
    fb3 = wp_shapes_only['fb3']

    out_T = nc.dram_tensor("out_T", [1, B], F32, kind="ExternalOutput")

    with tile.TileContext(nc) as tc:
        with tc.tile_pool(name="cst", bufs=1) as cst, \
             tc.tile_pool(name="state", bufs=1) as stt, \
             tc.tile_pool(name="wk", bufs=3) as wk, \
             tc.tile_pool(name="wk2", bufs=2) as wk2, \
             tc.tile_pool(name="ps", bufs=2, space="PSUM") as ps, \
             tc.tile_pool(name="dram", bufs=1, space="DRAM") as dr:
            # ---- load constants ----
            def load(pool, src, shape, dt):
                t = pool.tile(list(shape), dt, name=f"ld_{src.name}",
                              tag=f"ld_{src.name}")
                nc.sync.dma_start(t[:], src[:])
                return t

            iota_t = load(cst, iota_d, (128, 128), F32)
            I128 = load(cst, I128_d, (128, 128), BF16)
            bones = load(cst, bones_d, (H, HC), BF16)
            idxg_t = load(cst, idx_g, (128, EPC // 16), I16)
            idxr_t = load(cst, idx_xr, (128, EPC // 16), I16)
            dstrel_t = load(cst, dstrel_d, (128, NTILES), F32)
            xT_t = load(cst, x_T, (4, NPC), BF16)

            hT = stt.tile([128, NPC], BF16, tag="hT")
            zT = stt.tile([128, NPC], BF16, tag="zT")
            xr_tab = stt.tile([128, WINS, 128], BF16, tag="xrtab")  # row table
            xlrows = stt.tile([128, WINS, 128], BF16, tag="xlrows")

            agin = dr.tile([NPC, HC], BF16)
            xl_tab = dr.tile([NPAD, HC], BF16)
            xl_hi = dr.tile([NPAD - HALF, HC], BF16)
            st_in = dr.tile([128, 2], F32)
            st_out = dr.tile([128, 2], F32)
            pool_in = dr.tile([128, B], F32)
            pool_out = dr.tile([128, B], F32)

            # ---- node_embed: h0_T = relu(Wemb.T @ x_T) ----
            Wemb_t = load(cst, Wemb, (4, 64), BF16)
            bemb_t = load(cst, bemb, (64, 1), F32)
            for k in range(NPC // 512):
                pm = ps.tile([128, 512], F32, tag="m")
                nc.tensor.matmul(pm[:64, :], Wemb_t[:], xT_t[:, 512 * k:512 * (k + 1)],
                                 start=True, stop=True)
                nc.scalar.activation(hT[:64, 512 * k:512 * (k + 1)], pm[:64, :],
                                     mybir.ActivationFunctionType.Relu,
                                     bias=bemb_t[:], scale=1.0)

            # ---- layers ----
            for l in range(4):
                ind = 64 if l == 0 else 128
                if l == 0:
                    wlr = load(wk2, Wlr0, (64, 256), BF16)
                else:
                    wlr = wk2.tile([128, 256], BF16, tag="wlr")
                    nc.sync.dma_start(wlr[:], WlrR[l - 1])
                eW2_t = wk2.tile([2, HC], BF16, tag="ew2")
                nc.sync.dma_start(eW2_t[:], eW2_d[l])
                attb_t = wk2.tile([HC, H], BF16, tag="attb")
                nc.sync.dma_start(attb_t[:], attblk_d[l])
                gam_t = wk2.tile([HC, 1], F32, tag="gam")
                nc.sync.dma_start(gam_t[:], gamma_d[l])
                bet_t = wk2.tile([HC, 1], F32, tag="bet")
                nc.sync.dma_start(bet_t[:], beta_d[l])

                # transforms: per node-window rank
                for wn in range(WINS):
                    pt = ps.tile([128, 256], F32, tag="m")
                    nc.tensor.matmul(pt[:], hT[:ind, 128 * wn:128 * (wn + 1)],
                                     wlr[:ind, :], start=True, stop=True)
                    nc.vector.tensor_copy(xlrows[:, wn, :], pt[:, 0:128])
                    nc.vector.tensor_copy(xr_tab[:, wn, :], pt[:, 128:256])
                nc.sync.dma_start(
                    agin[:].rearrange("(r p) c -> p r c", p=128), xlrows[:])
                nc.gpsimd.collective_compute(
                    "AllGather", mybir.AluOpType.bypass, replica_groups=RG,
                    ins=[agin.opt()], outs=[xl_tab.opt()])
                nc.sync.dma_start(xl_hi[:], xl_tab[HALF:, :])

                # ---- edge phase ----
                for wn in range(WINS):
                    base = wn * TW * 128
                    pwin = ps.tile([128, 136], F32, tag="win")
                    for (t0, nt) in chunks:
                        is_hi = t0 >= TL
                        n = nt * 128
                        s0 = base + t0 * 128
                        wt0 = wn * TW + t0
                        gsrc = xl_hi if is_hi else xl_tab
                        xlT = wk.tile([128, 1, 512], BF16, tag="xlT")
                        nc.gpsimd.dma_gather(
                            xlT[:, :, :n], gsrc[:], idxg_t[:, s0 // 16:(s0 + n) // 16],
                            n, n, 128, transpose=True,
                            queue_num=(wn % 3))
                        ohe = wk.tile([128, 4, 128], BF16, tag="ohe")
                        nc.sync.dma_start(
                            ohe[:, :nt, :],
                            oh_en_d[wt0:wt0 + nt].rearrange("t p j -> p t j"))
                        ohn = wk.tile([128, 4, 128], BF16, tag="ohn")
                        nc.sync.dma_start(
                            ohn[:, :nt, :],
                            oh_ne_d[wt0:wt0 + nt].rearrange("t p j -> p t j"))
                        ea_t = wk.tile([2, 512], BF16, tag="ea")
                        nc.sync.dma_start(ea_t[:, :n], ea2_d[:, s0:s0 + n])

                        # m = e(+biases) + xr-broadcast + xl  in PSUM
                        pm = ps.tile([128, 512], F32, tag="m")
                        nc.tensor.matmul(pm[:, :n], eW2_t[:], ea_t[:, :n],
                                         start=True, stop=False)
                        nc.tensor.matmul(pm[:, :n], xr_tab[:, wn, :],
                                         ohn[:].rearrange("p t j -> p (t j)")[:, :n],
                                         start=False, stop=False)
                        nc.tensor.matmul(pm[:, :n], I128[:], xlT[:, 0, :n],
                                         start=False, stop=True)
                        # value rows: G = transpose(xlT) via PE
                        pg = ps.tile([128, 4, 128], F32, tag="g")
                        for ti in range(nt):
                            nc.tensor.matmul(pg[:, ti, :],
                                             xlT[:, 0, 128 * ti:128 * (ti + 1)],
                                             I128[:], start=True, stop=True)
                        G = wk.tile([128, 4, 128], BF16, tag="G")
                        nc.scalar.activation(
                            G[:].rearrange("p t c -> p (t c)")[:, :n],
                            pg[:].rearrange("p t c -> p (t c)")[:, :n],
                            mybir.ActivationFunctionType.Identity)
                        t_sb = wk.tile([128, 512], BF16, tag="t")
                        nc.scalar.activation(t_sb[:, :n], pm[:, :n],
                                             mybir.ActivationFunctionType.Prelu,
                                             alpha=NEG)
                        pss = ps.tile([8, 512], F32, tag="s", bufs=1)
                        nc.tensor.matmul(pss[:, :n], attb_t[:], t_sb[:, :n],
                                         start=True, stop=True)
                        ex = wk.tile([8, 512], BF16, tag="ex")
                        nc.scalar.activation(ex[:, :n], pss[:, :n],
                                             mybir.ActivationFunctionType.Exp)
                        pex = ps.tile([128, 4, 128], F32, tag="ex128", bufs=1)
                        for ti in range(nt):
                            nc.tensor.matmul(pex[:, ti, :],
                                             ex[:, 128 * ti:128 * (ti + 1)],
                                             bones[:], start=True, stop=True)
                        aG = wk.tile([128, 4, 136], BF16, tag="aG")
                        nc.vector.tensor_mul(
                            aG[:, :nt, 0:128], G[:, :nt, :], pex[:, :nt, :])
                        nc.vector.tensor_copy(
                            aG[:, :nt, 128:136],
                            pex[:, :nt, :].rearrange("p t (h s) -> p t h s", s=16)[:, :, :, 0])
                        for ti in range(nt):
                            gt = t0 + ti
                            nc.tensor.matmul(pwin[:], ohe[:, ti, :], aG[:, ti, :],
                                             start=(gt == 0), stop=(gt == TW - 1))
                    # normalize window: z = num / (den + eps)
                    den = wk.tile([128, 8], F32, tag="den")
                    nc.vector.tensor_scalar(den[:], pwin[:, 128:136], DEN_EPS, None,
                                            mybir.AluOpType.add)
                    rden = wk.tile([128, 8], F32, tag="rden")
                    nc.vector.reciprocal(rden[:], den[:])
                    zB = wk.tile([128, 128], BF16, tag="zB")
                    nc.vector.tensor_mul(
                        zB[:].rearrange("p (h s) -> p h s", s=16),
                        pwin[:, 0:128].rearrange("p (h s) -> p h s", s=16),
                        rden[:].to_broadcast([128, 8, 16]))
                    pzt = ps.tile([128, 128], F32, tag="s", bufs=1)
                    nc.tensor.matmul(pzt[:], zB[:], I128[:], start=True, stop=True)
                    nc.vector.tensor_copy(zT[:, 128 * wn:128 * (wn + 1)], pzt[:])

                # ---- BN stats ----
                zsum = wk2.tile([128, NPC // 512], F32, tag="zsum")
                zsq = wk2.tile([128, NPC // 512], F32, tag="zsq")
                trash = wk2.tile([128, 512], BF16, tag="trash")
                for k in range(NPC // 512):
                    ch = zT[:, 512 * k:512 * (k + 1)]
                    nc.vector.tensor_reduce(zsum[:, k:k + 1], ch,
                                            mybir.AxisListType.X, mybir.AluOpType.add)
                    nc.scalar.activation(trash[:], ch,
                                         mybir.ActivationFunctionType.Square,
                                         accum_out=zsq[:, k:k + 1])
                stats = wk2.tile([128, 2], F32, tag="stats")
                nc.vector.tensor_reduce(stats[:, 0:1], zsum[:],
                                        mybir.AxisListType.X, mybir.AluOpType.add)
                nc.vector.tensor_reduce(stats[:, 1:2], zsq[:],
                                        mybir.AxisListType.X, mybir.AluOpType.add)
                nc.gpsimd.dma_start(st_in[:], stats[:])
                nc.gpsimd.collective_compute(
                    "AllReduce", mybir.AluOpType.add, replica_groups=RG,
                    ins=[st_in.opt()], outs=[st_out.opt()])
                sg = wk2.tile([128, 2], F32, tag="sg")
                nc.sync.dma_start(sg[:], st_out[:])
                mu = wk2.tile([128, 1], F32, tag="mu")
                nc.vector.tensor_scalar(mu[:], sg[:, 0:1], 1.0 / N, None,
                                        mybir.AluOpType.mult)
                ez2 = wk2.tile([128, 1], F32, tag="ez2")
                nc.vector.tensor_scalar(ez2[:], sg[:, 1:2], 1.0 / N, None,
                                        mybir.AluOpType.mult)
                mu2 = wk2.tile([128, 1], F32, tag="mu2")
                nc.vector.tensor_mul(mu2[:], mu[:], mu[:])
                var = wk2.tile([128, 1], F32, tag="var")
                nc.vector.tensor_sub(var[:], ez2[:], mu2[:])
                nc.vector.tensor_scalar(var[:], var[:], EPS, None,
                                        mybir.AluOpType.add)
                sd = wk2.tile([128, 1], F32, tag="sd")
                nc.scalar.activation(sd[:], var[:], mybir.ActivationFunctionType.Sqrt)
                rsd = wk2.tile([128, 1], F32, tag="rsd")
                nc.vector.reciprocal(rsd[:], sd[:])
                gh = wk2.tile([128, 1], F32, tag="gh")
                nc.vector.tensor_mul(gh[:], gam_t[:], rsd[:])
                mgh = wk2.tile([128, 1], F32, tag="mgh")
                nc.vector.tensor_mul(mgh[:], mu[:], gh[:])
                bh = wk2.tile([128, 1], F32, tag="bh")
                nc.vector.tensor_sub(bh[:], bet_t[:], mgh[:])

                # ---- h = relu(gh*z + bh (+res)) ----
                for k in range(NPC // 512):
                    zch = zT[:, 512 * k:512 * (k + 1)]
                    hch = hT[:, 512 * k:512 * (k + 1)]
                    if l == 0:
                        nc.scalar.activation(hch, zch,
                                             mybir.ActivationFunctionType.Relu,
                                             bias=bh[:], scale=gh[:])
                    else:
                        tmp = wk.tile([128, 512], BF16, tag="tmp")
                        nc.vector.scalar_tensor_tensor(
                            tmp[:], zch, gh[:], hch,
                            mybir.AluOpType.mult, mybir.AluOpType.add)
                        nc.scalar.activation(hch, tmp[:],
                                             mybir.ActivationFunctionType.Relu,
                                             bias=bh[:], scale=1.0)

            # ---- pooling: pooled_T[c, g] = sum_n h_rows[n, c] * S[n, g] ----
            S_t = cst.tile([128, WINS, B], BF16)
            nc.sync.dma_start(S_t[:], S_d[:].rearrange("(r p) g -> p r g", p=128))
            ppool = ps.tile([128, B], F32, tag="win")
            for wn in range(WINS):
                phr = ps.tile([128, 128], F32, tag="s", bufs=1)
                nc.tensor.matmul(phr[:], hT[:, 128 * wn:128 * (wn + 1)], I128[:],
                                 start=True, stop=True)
                hrow = wk.tile([128, 128], BF16, tag="hrow")
                nc.vector.tensor_copy(hrow[:], phr[:])
                nc.tensor.matmul(ppool[:], hrow[:], S_t[:, wn, :],
                                 start=(wn == 0), stop=(wn == WINS - 1))
            poolp = wk2.tile([128, B], F32, tag="poolp")
            nc.vector.tensor_copy(poolp[:], ppool[:])
            nc.gpsimd.dma_start(pool_in[:], poolp[:])
            nc.gpsimd.collective_compute(
                "AllReduce", mybir.AluOpType.add, replica_groups=RG,
                ins=[pool_in.opt()], outs=[pool_out.opt()])
            poolT = wk2.tile([128, B], F32, tag="poolT")
            nc.sync.dma_start(poolT[:], pool_out[:])

            # ---- head (fp32, replicated) ----
            gf_t = load(cst, gf_T, (6, B), F32)
            gfb_t = cst.tile([2, B], F32, name="gfb", tag="gfb")
            nc.sync.dma_start(gfb_t[:], gf_T[4:6, :])
            mW1_t = load(cst, mW1, (4, 16), F32); mb1_t = load(cst, mb1, (16, 1), F32)
            mW2_t = load(cst, mW2, (16, 16), F32); mb2_t = load(cst, mb2, (16, 1), F32)
            gW1_t = load(cst, gW1, (2, 16), F32); gb1_t = load(cst, gb1, (16, 1), F32)
            gW2_t = load(cst, gW2, (16, 16), F32); gb2_t = load(cst, gb2, (16, 1), F32)
            fW1a_t = load(cst, fW1a, (128, 128), F32)
            fW1bm_t = load(cst, fW1bm, (16, 128), F32)
            fW1bg_t = load(cst, fW1bg, (16, 128), F32)
            fb1_t = load(cst, fb1, (128, 1), F32)
            fW2_t = load(cst, fW2, (128, 64), F32); fb2_t = load(cst, fb2, (64, 1), F32)
            fW3_t = load(cst, fW3, (64, 1), F32)

            meta_t = wk2.tile([16, B], F32, tag="meta_t")
            graph_t = wk2.tile([16, B], F32, tag="graph_t")
            pm1 = ps.tile([16, B], F32, tag="s", bufs=1)
            nc.tensor.matmul(pm1[:], mW1_t[:], gf_t[0:4, :], start=True, stop=True)
            m1h = wk2.tile([16, B], F32, tag="m1h")
            nc.scalar.activation(m1h[:], pm1[:], mybir.ActivationFunctionType.Relu,
                                 bias=mb1_t[:])
            pm2 = ps.tile([16, B], F32, tag="s", bufs=1)
            nc.tensor.matmul(pm2[:], mW2_t[:], m1h[:], start=True, stop=True)
            nc.scalar.activation(meta_t[:], pm2[:],
                                 mybir.ActivationFunctionType.Relu, bias=mb2_t[:])
            pg1 = ps.tile([16, B], F32, tag="s", bufs=1)
            nc.tensor.matmul(pg1[:], gW1_t[:], gfb_t[:], start=True, stop=True)
            g1h = wk2.tile([16, B], F32, tag="g1h")
            nc.scalar.activation(g1h[:], pg1[:], mybir.ActivationFunctionType.Relu,
                                 bias=gb1_t[:])
            pg2 = ps.tile([16, B], F32, tag="s", bufs=1)
            nc.tensor.matmul(pg2[:], gW2_t[:], g1h[:], start=True, stop=True)
            nc.scalar.activation(graph_t[:], pg2[:],
                                 mybir.ActivationFunctionType.Relu, bias=gb2_t[:])

            pf1 = ps.tile([128, B], F32, tag="m")
            nc.tensor.matmul(pf1[:], fW1a_t[:], poolT[:], start=True, stop=False)
            nc.tensor.matmul(pf1[:], fW1bm_t[:], meta_t[:], start=False, stop=False)
            nc.tensor.matmul(pf1[:], fW1bg_t[:], graph_t[:], start=False, stop=True)
            f1 = wk2.tile([128, B], F32, tag="f1")
            nc.scalar.activation(f1[:], pf1[:], mybir.ActivationFunctionType.Relu,
                                 bias=fb1_t[:])
            pf2 = ps.tile([64, B], F32, tag="s", bufs=1)
            nc.tensor.matmul(pf2[:], fW2_t[:], f1[:], start=True, stop=True)
            f2 = wk2.tile([64, B], F32, tag="f2")
            nc.scalar.activation(f2[:], pf2[:], mybir.ActivationFunctionType.Relu,
                                 bias=fb2_t[:])
            pf3 = ps.tile([1, B], F32, tag="s", bufs=1)
            nc.tensor.matmul(pf3[:], fW3_t[:], f2[:], start=True, stop=True)
            fout = wk2.tile([1, B], F32, tag="fout")
            nc.scalar.activation(fout[:], pf3[:],
                                 mybir.ActivationFunctionType.Identity, bias=fb3)
            nc.sync.dma_start(out_T[:], fout[:])

    nc.compile()
    return nc


def kernel(**inputs):
    x = np.asarray(inputs['x'], np.float32)
    edge_index = np.asarray(inputs['edge_index']).astype(np.int64)
    edge_attr = np.asarray(inputs['edge_attr'], np.float32)
    batch = np.asarray(inputs['batch']).astype(np.int64)
    gfeat = np.asarray(inputs['global_features'], np.float32)

    per_core, st = preprocess(x, edge_index, edge_attr, batch, gfeat)
    ws = {k: np.asarray(v, np.float32) for k, v in inputs.items()
          if k not in ('x', 'edge_index', 'edge_attr', 'batch', 'global_features')}
    wp = pack_weights(ws)

    nc = build(st, wp)

    gf_T = np.ascontiguousarray(gfeat[:, 0, :].T).astype(np.float32)
    wmap = {k: v for k, v in wp.items() if k != 'fb3'}
    in_maps = []
    for c in range(NCORES):
        m = dict(per_core[c])
        m['gf_T'] = gf_T
        m.update(wmap)
        in_maps.append(m)

    res = bass_utils.run_bass_kernel_spmd(
        nc, in_maps, core_ids=list(range(NCORES)), trace=False)
    out = res.results[0]['out_T']            # [1, 100]
    return np.ascontiguousarray(out.reshape(B, 1)).astype(np.float32)


if __name__ == "__main__":
    data = dict(np.load('/root/problem/inputs_cache.npz'))
    out = kernel(**data)
    exp = np.load('/root/problem/expected_cpu.npy')
    err = np.abs(out - exp).max() / np.abs(exp).max()
    print("Relative error:", err)
